# revision 7
# baseline (speedup 1.0000x reference)
"""AveragePrevEmbeddingsLM Trainium2 kernel (8 NeuronCores, vocab-sharded).

logits[b, t, v] = mean(emb_table[x[b, :t+1]]) @ W.T + b_vec

Strategy: shard the vocab dim across 8 cores (4000 each). Every core
redundantly gathers + prefix-sums all 8192 token embeddings (cheap),
then computes its (8192 x 64) @ (64 x 4000) logits slice. The 1 GB
logits write is the memory roofline (~131 MB/core).

Device pipeline per core:
  dma_gather (emb rows, per batch)  -> [128tok, 16blk, 64emb] SBUF
  PE transpose per 128-token block  -> [64emb, 128tok] PSUM -> SBUF seg
  tensor_tensor_scan along seq      -> causal prefix sums Y
  per 128-token tile: matmul(lhsT=[Y; pos+1], rhs=[W.T; bias]) -> PSUM
  ScalarE scaled copy (x 1/(pos+1)) -> SBUF -> 2MB DMA out

The bias is folded in via an extra contraction row (lhsT row 64 =
pos+1, rhs row 64 = bias); dividing by pos+1 on the PSUM->SBUF copy
then yields mean-pooled logits + bias exactly.
"""

import os
import sys

import numpy as np

for _p in ("/opt/trn_rl_repo",):
    if _p not in sys.path and os.path.isdir(_p):
        sys.path.append(_p)

VOCAB, EMB, B, SEQ = 32000, 64, 4, 2048
NCORES = 8
VS = VOCAB // NCORES       # vocab shard per core
TOK = B * SEQ
BLK = SEQ // 128           # 128-token blocks per batch row
MTILES = TOK // 128
NCHUNK = 8
CHUNK = VS // NCHUNK       # matmul free-dim chunk (one PSUM bank)

COMPUTE = os.environ.get("KERNEL_COMPUTE", "f32r")   # f32r | f32 | bf16
K_ROWS = int(os.environ.get("KERNEL_K_ROWS", "65"))  # 65 (exact) or 128 (padded)
OUT_DT = os.environ.get("KERNEL_OUT_DT", "bf16")     # bf16 | f32 output logits

_prog_cache = {}


def _build(compute: str, k_rows: int, out_dt: str):
    from concourse import bacc
    import concourse.mybir as mybir
    import concourse.tile as tile
    from concourse.masks import make_identity

    f32 = mybir.dt.float32
    cdt = {
        "f32r": mybir.dt.float32r,
        "f32": f32,
        "bf16": mybir.dt.bfloat16,
    }[compute]
    odt = {"bf16": mybir.dt.bfloat16, "f32": f32}[out_dt]

    nc = bacc.Bacc(None, target_bir_lowering=False)

    emb_d = nc.dram_tensor("emb", [VOCAB, EMB], f32, kind="ExternalInput")
    idx_d = nc.dram_tensor("idx", [128, TOK // 128], mybir.dt.int32, kind="ExternalInput")
    wtb_d = nc.dram_tensor("wtb", [128, VS], f32, kind="ExternalInput")
    posp1_d = nc.dram_tensor("posp1", [1, SEQ], f32, kind="ExternalInput")
    recip_d = nc.dram_tensor("recip", [128, BLK], f32, kind="ExternalInput")
    out_d = nc.dram_tensor("out", [TOK, VS], odt, kind="ExternalOutput")

    with tile.TileContext(nc) as tc:
        with (
            tc.tile_pool(name="const", bufs=1) as constp,
            tc.tile_pool(name="gath", bufs=2) as gathp,
            tc.tile_pool(name="segraw", bufs=2) as segrawp,
            tc.tile_pool(name="segcum", bufs=2) as segcump,
            tc.tile_pool(name="outp", bufs=6) as outp,
            tc.tile_pool(name="ptr", bufs=1, space="PSUM") as ptrp,
            tc.tile_pool(name="pmm", bufs=7, space="PSUM") as pmmp,
        ):
            wtb_sb = constp.tile([128, VS], f32)
            nc.sync.dma_start(wtb_sb[:], wtb_d[:])
            recip_sb = constp.tile([128, BLK], f32)
            nc.sync.dma_start(recip_sb[:], recip_d[:])
            idx_sb = constp.tile([128, TOK // 128], mybir.dt.int32)
            nc.sync.dma_start(idx_sb[:], idx_d[:])
            ident = constp.tile([128, 128], f32)
            make_identity(nc, ident[:])

            if cdt == f32:
                wtb_c = wtb_sb[:]
            else:
                wtb_cast = constp.tile([128, VS], cdt)
                nc.vector.tensor_copy(wtb_cast[:], wtb_sb[:])
                wtb_c = wtb_cast[:]

            import concourse.bass as bass

            # Software pipeline at 512-token (4 m-tile) "quarter"
            # granularity: head(Q) = gather + PE-transpose + chained scan
            # (+ cast); proj(Q) = 4 m-tiles of matmul + scaled copy + DMA
            # out. head(Q+1) is emitted before proj(Q) so each engine's
            # in-order stream interleaves next-quarter prep with current
            # projections.
            QT = 4                      # m-tiles per quarter
            NQ = MTILES // QT           # total quarters (16)
            QSEQ = QT * 128             # tokens per quarter (512)
            state = {}

            def head(Q):
                b, q = Q // (BLK // QT), Q % (BLK // QT)
                if q == 0:
                    state["gath"] = gathp.tile([128, BLK, EMB], f32, tag="gath", name="gath")
                    state["seg_raw"] = segrawp.tile([EMB, SEQ], f32, tag="seg_raw", name="seg_raw")
                    state["seg_cum"] = segcump.tile([k_rows, SEQ], f32, tag="seg_cum", name="seg_cum")
                    nc.sync.dma_start(
                        state["seg_cum"][EMB:EMB + 1, :], posp1_d[:])
                    if k_rows > EMB + 1:
                        nc.vector.memset(
                            state["seg_cum"][EMB + 1:k_rows, :], 0.0)
                    if cdt != f32:
                        state["seg_cast"] = segcump.tile(
                            [k_rows, SEQ], cdt, tag="segcast", name="segcast")
                        if k_rows > EMB:
                            nc.vector.tensor_copy(
                                state["seg_cast"][EMB:k_rows, :],
                                state["seg_cum"][EMB:k_rows, :])
                gath, seg_raw = state["gath"], state["seg_raw"]
                seg_cum = state["seg_cum"]
                for mb in range(q * QT, (q + 1) * QT):
                    m = b * BLK + mb
                    nc.gpsimd.indirect_dma_start(
                        out=gath[:, mb, :],
                        out_offset=None,
                        in_=emb_d[:],
                        in_offset=bass.IndirectOffsetOnAxis(
                            ap=idx_sb[:, m:m + 1], axis=0,
                        ),
                    )
                    pt = ptrp.tile([EMB, 128], f32)
                    nc.tensor.transpose(pt[:], gath[:, mb, :], ident[:])
                    nc.vector.tensor_copy(
                        seg_raw[:, mb * 128:(mb + 1) * 128], pt[:])
                qsl = slice(q * QSEQ, (q + 1) * QSEQ)
                initial = (0.0 if q == 0 else
                           seg_cum[0:EMB, q * QSEQ - 1:q * QSEQ])
                nc.vector.tensor_tensor_scan(
                    seg_cum[0:EMB, qsl],
                    seg_raw[0:EMB, qsl],
                    seg_raw[0:EMB, qsl],
                    initial,
                    op0=mybir.AluOpType.add,
                    op1=mybir.AluOpType.bypass,
                )
                if cdt != f32:
                    nc.vector.tensor_copy(
                        state["seg_cast"][0:EMB, qsl], seg_cum[0:EMB, qsl])
                    state["seg_c"] = state["seg_cast"][:]
                else:
                    state["seg_c"] = seg_cum[:]

            def proj(Q, seg_c):
                b, q = Q // (BLK // QT), Q % (BLK // QT)
                for mb in range(q * QT, (q + 1) * QT):
                    m = b * BLK + mb
                    otile = outp.tile([128, NCHUNK, CHUNK], odt)
                    lhsT = seg_c[:, mb * 128:(mb + 1) * 128]
                    scale = recip_sb[:, mb:mb + 1]
                    # 8 single-bank PSUM tiles (bank = 512 f32), one
                    # N=500 matmul each, then per-chunk scaled copy,
                    # alternating ACT/DVE.
                    for ch in range(NCHUNK):
                        ps = pmmp.tile([128, 512], f32)
                        nc.tensor.matmul(
                            ps[:, 0:CHUNK],
                            lhsT,
                            wtb_c[0:k_rows, ch * CHUNK:(ch + 1) * CHUNK],
                            start=True,
                            stop=True,
                        )
                        osl = otile[:, ch, :]
                        if ch % 8 != 1 and ch % 8 != 4 and ch % 8 != 6:
                            nc.scalar.activation(
                                osl, ps[:, 0:CHUNK],
                                mybir.ActivationFunctionType.Copy,
                                scale=scale,
                            )
                        else:
                            nc.vector.tensor_scalar_mul(
                                osl, ps[:, 0:CHUNK], scale)
                        if ch == 3:
                            nc.sync.dma_start(
                                out_d[m * 128:(m + 1) * 128, 0:VS // 2],
                                otile[:, 0:NCHUNK // 2, :])
                        elif ch == NCHUNK - 1:
                            nc.sync.dma_start(
                                out_d[m * 128:(m + 1) * 128, VS // 2:VS],
                                otile[:, NCHUNK // 2:NCHUNK, :])


            LEAD = 1
            seg_of = {}
            for Q in range(min(LEAD, NQ)):
                head(Q)
                seg_of[Q] = state["seg_c"]
            for Q in range(NQ):
                if Q + LEAD < NQ:
                    head(Q + LEAD)
                    seg_of[Q + LEAD] = state["seg_c"]
                proj(Q, seg_of.pop(Q))

    nc.compile()
    return nc


def _get_prog(compute: str, k_rows: int, out_dt: str):
    key = (compute, k_rows, out_dt)
    if key not in _prog_cache:
        _prog_cache[key] = _build(compute, k_rows, out_dt)
    return _prog_cache[key]


def _make_in_maps(emb_table, W, b, x):
    emb_table = np.ascontiguousarray(np.asarray(emb_table, dtype=np.float32))
    W = np.asarray(W, dtype=np.float32)
    b = np.asarray(b, dtype=np.float32)
    x = np.asarray(x).astype(np.int64).reshape(B, SEQ)

    # idx layout: token m*128 + p -> idx[p, m]
    wrapped = np.ascontiguousarray(
        x.reshape(-1).reshape(TOK // 128, 128).T.astype(np.int32)
    )

    posp1 = np.arange(1, SEQ + 1, dtype=np.float32)[None, :]
    i = np.arange(128)[:, None]
    mb = np.arange(BLK)[None, :]
    recip = (1.0 / (mb * 128 + i + 1)).astype(np.float32)

    in_maps = []
    for c in range(NCORES):
        wtb = np.zeros((128, VS), dtype=np.float32)
        wtb[0:EMB] = W[c * VS:(c + 1) * VS, :].T
        wtb[EMB] = b[c * VS:(c + 1) * VS]
        in_maps.append({
            "emb": emb_table,
            "idx": wrapped,
            "wtb": np.ascontiguousarray(wtb),
            "posp1": posp1,
            "recip": recip,
        })
    return in_maps


def kernel(emb_table, W, b, x, trace=False):
    from concourse.bass_utils import run_bass_kernel_spmd

    nc = _get_prog(COMPUTE, K_ROWS, OUT_DT)
    in_maps = _make_in_maps(emb_table, W, b, x)
    res = run_bass_kernel_spmd(
        nc, in_maps, core_ids=list(range(NCORES)), trace=trace,
    )

    out = np.empty((TOK, VOCAB), dtype=np.float32)
    for c in range(NCORES):
        out[:, c * VS:(c + 1) * VS] = np.asarray(
            res.results[c]["out"]).astype(np.float32)
    out = out.reshape(B, SEQ, VOCAB)
    if trace:
        return out, res
    return out



# revision 8
# speedup vs baseline: 1.2346x; 1.2346x over previous
"""AveragePrevEmbeddingsLM Trainium2 kernel (8 NeuronCores, vocab-sharded).

logits[b, t, v] = mean(emb_table[x[b, :t+1]]) @ W.T + b_vec

Strategy: shard the vocab dim across 8 cores (4000 each). Every core
redundantly gathers + prefix-sums all 8192 token embeddings (cheap),
then computes its (8192 x 64) @ (64 x 4000) logits slice. The 1 GB
logits write is the memory roofline (~131 MB/core).

Device pipeline per core:
  dma_gather (emb rows, per batch)  -> [128tok, 16blk, 64emb] SBUF
  PE transpose per 128-token block  -> [64emb, 128tok] PSUM -> SBUF seg
  tensor_tensor_scan along seq      -> causal prefix sums Y
  per 128-token tile: matmul(lhsT=[Y; pos+1], rhs=[W.T; bias]) -> PSUM
  ScalarE scaled copy (x 1/(pos+1)) -> SBUF -> 2MB DMA out

The bias is folded in via an extra contraction row (lhsT row 64 =
pos+1, rhs row 64 = bias); dividing by pos+1 on the PSUM->SBUF copy
then yields mean-pooled logits + bias exactly.
"""

import os
import sys

import numpy as np

for _p in ("/opt/trn_rl_repo",):
    if _p not in sys.path and os.path.isdir(_p):
        sys.path.append(_p)

VOCAB, EMB, B, SEQ = 32000, 64, 4, 2048
NCORES = 8
VS = VOCAB // NCORES       # vocab shard per core
TOK = B * SEQ
BLK = SEQ // 128           # 128-token blocks per batch row
MTILES = TOK // 128
NCHUNK = 8
CHUNK = VS // NCHUNK       # matmul free-dim chunk (one PSUM bank)

COMPUTE = os.environ.get("KERNEL_COMPUTE", "f32r")   # f32r | f32 | bf16
K_ROWS = int(os.environ.get("KERNEL_K_ROWS", "65"))  # 65 (exact) or 128 (padded)
OUT_DT = os.environ.get("KERNEL_OUT_DT", "fp8")      # fp8 | bf16 | f32 logits

_prog_cache = {}


def _g_scale():
    """Per-position power-of-2 normalizer: the pooled (pre-bias) logit at
    position t has std ~0.577/sqrt(t+1); scale it to ~unit range so the
    e3m4 output stays in the normal range. Host divides it back out."""
    t = np.arange(SEQ, dtype=np.float64)
    return (2.0 ** np.round(np.log2(np.sqrt(t + 1) / 0.577))).astype(np.float32)


def _build(compute: str, k_rows: int, out_dt: str):
    from concourse import bacc
    import concourse.mybir as mybir
    import concourse.tile as tile
    from concourse.masks import make_identity

    f32 = mybir.dt.float32
    cdt = {
        "f32r": mybir.dt.float32r,
        "f32": f32,
        "bf16": mybir.dt.bfloat16,
    }[compute]
    odt = {"bf16": mybir.dt.bfloat16, "f32": f32}[out_dt]

    nc = bacc.Bacc(None, target_bir_lowering=False)

    emb_d = nc.dram_tensor("emb", [VOCAB, EMB], f32, kind="ExternalInput")
    idx_d = nc.dram_tensor("idx", [128, TOK // 128], mybir.dt.int32, kind="ExternalInput")
    wtb_d = nc.dram_tensor("wtb", [128, VS], f32, kind="ExternalInput")
    posp1_d = nc.dram_tensor("posp1", [1, SEQ], f32, kind="ExternalInput")
    recip_d = nc.dram_tensor("recip", [128, BLK], f32, kind="ExternalInput")
    out_d = nc.dram_tensor("out", [TOK, VS], odt, kind="ExternalOutput")

    with tile.TileContext(nc) as tc:
        with (
            tc.tile_pool(name="const", bufs=1) as constp,
            tc.tile_pool(name="gath", bufs=2) as gathp,
            tc.tile_pool(name="segraw", bufs=2) as segrawp,
            tc.tile_pool(name="segcum", bufs=2) as segcump,
            tc.tile_pool(name="outp", bufs=6) as outp,
            tc.tile_pool(name="ptr", bufs=1, space="PSUM") as ptrp,
            tc.tile_pool(name="pmm", bufs=7, space="PSUM") as pmmp,
        ):
            wtb_sb = constp.tile([128, VS], f32)
            nc.sync.dma_start(wtb_sb[:], wtb_d[:])
            recip_sb = constp.tile([128, BLK], f32)
            nc.sync.dma_start(recip_sb[:], recip_d[:])
            idx_sb = constp.tile([128, TOK // 128], mybir.dt.int32)
            nc.sync.dma_start(idx_sb[:], idx_d[:])
            ident = constp.tile([128, 128], f32)
            make_identity(nc, ident[:])

            if cdt == f32:
                wtb_c = wtb_sb[:]
            else:
                wtb_cast = constp.tile([128, VS], cdt)
                nc.vector.tensor_copy(wtb_cast[:], wtb_sb[:])
                wtb_c = wtb_cast[:]

            import concourse.bass as bass

            # Software pipeline at 512-token (4 m-tile) "quarter"
            # granularity: head(Q) = gather + PE-transpose + chained scan
            # (+ cast); proj(Q) = 4 m-tiles of matmul + scaled copy + DMA
            # out. head(Q+1) is emitted before proj(Q) so each engine's
            # in-order stream interleaves next-quarter prep with current
            # projections.
            QT = 4                      # m-tiles per quarter
            NQ = MTILES // QT           # total quarters (16)
            QSEQ = QT * 128             # tokens per quarter (512)
            state = {}

            def head(Q):
                b, q = Q // (BLK // QT), Q % (BLK // QT)
                if q == 0:
                    state["gath"] = gathp.tile([128, BLK, EMB], f32, tag="gath", name="gath")
                    state["seg_raw"] = segrawp.tile([EMB, SEQ], f32, tag="seg_raw", name="seg_raw")
                    state["seg_cum"] = segcump.tile([k_rows, SEQ], f32, tag="seg_cum", name="seg_cum")
                    nc.sync.dma_start(
                        state["seg_cum"][EMB:EMB + 1, :], posp1_d[:])
                    if k_rows > EMB + 1:
                        nc.vector.memset(
                            state["seg_cum"][EMB + 1:k_rows, :], 0.0)
                    if cdt != f32:
                        state["seg_cast"] = segcump.tile(
                            [k_rows, SEQ], cdt, tag="segcast", name="segcast")
                        if k_rows > EMB:
                            nc.vector.tensor_copy(
                                state["seg_cast"][EMB:k_rows, :],
                                state["seg_cum"][EMB:k_rows, :])
                gath, seg_raw = state["gath"], state["seg_raw"]
                seg_cum = state["seg_cum"]
                for mb in range(q * QT, (q + 1) * QT):
                    m = b * BLK + mb
                    nc.gpsimd.indirect_dma_start(
                        out=gath[:, mb, :],
                        out_offset=None,
                        in_=emb_d[:],
                        in_offset=bass.IndirectOffsetOnAxis(
                            ap=idx_sb[:, m:m + 1], axis=0,
                        ),
                    )
                    pt = ptrp.tile([EMB, 128], f32)
                    nc.tensor.transpose(pt[:], gath[:, mb, :], ident[:])
                    nc.vector.tensor_copy(
                        seg_raw[:, mb * 128:(mb + 1) * 128], pt[:])
                qsl = slice(q * QSEQ, (q + 1) * QSEQ)
                initial = (0.0 if q == 0 else
                           seg_cum[0:EMB, q * QSEQ - 1:q * QSEQ])
                nc.vector.tensor_tensor_scan(
                    seg_cum[0:EMB, qsl],
                    seg_raw[0:EMB, qsl],
                    seg_raw[0:EMB, qsl],
                    initial,
                    op0=mybir.AluOpType.add,
                    op1=mybir.AluOpType.bypass,
                )
                if cdt != f32:
                    nc.vector.tensor_copy(
                        state["seg_cast"][0:EMB, qsl], seg_cum[0:EMB, qsl])
                    state["seg_c"] = state["seg_cast"][:]
                else:
                    state["seg_c"] = seg_cum[:]

            def proj(Q, seg_c):
                b, q = Q // (BLK // QT), Q % (BLK // QT)
                for mb in range(q * QT, (q + 1) * QT):
                    m = b * BLK + mb
                    otile = outp.tile([128, NCHUNK, CHUNK], odt)
                    lhsT = seg_c[:, mb * 128:(mb + 1) * 128]
                    scale = recip_sb[:, mb:mb + 1]
                    # 8 single-bank PSUM tiles (bank = 512 f32), one
                    # N=500 matmul each, then per-chunk scaled copy,
                    # alternating ACT/DVE.
                    for ch in range(NCHUNK):
                        ps = pmmp.tile([128, 512], f32)
                        nc.tensor.matmul(
                            ps[:, 0:CHUNK],
                            lhsT,
                            wtb_c[0:k_rows, ch * CHUNK:(ch + 1) * CHUNK],
                            start=True,
                            stop=True,
                        )
                        osl = otile[:, ch, :]
                        if ch % 8 != 1 and ch % 8 != 4 and ch % 8 != 6:
                            nc.scalar.activation(
                                osl, ps[:, 0:CHUNK],
                                mybir.ActivationFunctionType.Copy,
                                scale=scale,
                            )
                        else:
                            nc.vector.tensor_scalar_mul(
                                osl, ps[:, 0:CHUNK], scale)
                        if ch == 3:
                            nc.sync.dma_start(
                                out_d[m * 128:(m + 1) * 128, 0:VS // 2],
                                otile[:, 0:NCHUNK // 2, :])
                        elif ch == NCHUNK - 1:
                            nc.sync.dma_start(
                                out_d[m * 128:(m + 1) * 128, VS // 2:VS],
                                otile[:, NCHUNK // 2:NCHUNK, :])


            LEAD = 1
            seg_of = {}
            for Q in range(min(LEAD, NQ)):
                head(Q)
                seg_of[Q] = state["seg_c"]
            for Q in range(NQ):
                if Q + LEAD < NQ:
                    head(Q + LEAD)
                    seg_of[Q + LEAD] = state["seg_c"]
                proj(Q, seg_of.pop(Q))

    nc.compile()
    return nc


def _get_prog(compute: str, k_rows: int, out_dt: str):
    key = (compute, k_rows, out_dt)
    if key not in _prog_cache:
        _prog_cache[key] = _build(compute, k_rows, out_dt)
    return _prog_cache[key]


def _make_in_maps(emb_table, W, b, x):
    emb_table = np.ascontiguousarray(np.asarray(emb_table, dtype=np.float32))
    W = np.asarray(W, dtype=np.float32)
    b = np.asarray(b, dtype=np.float32)
    x = np.asarray(x).astype(np.int64).reshape(B, SEQ)

    # idx layout: token m*128 + p -> idx[p, m]
    wrapped = np.ascontiguousarray(
        x.reshape(-1).reshape(TOK // 128, 128).T.astype(np.int32)
    )

    posp1 = np.arange(1, SEQ + 1, dtype=np.float32)[None, :]
    i = np.arange(128)[:, None]
    mb = np.arange(BLK)[None, :]
    recip = (1.0 / (mb * 128 + i + 1)).astype(np.float32)

    in_maps = []
    for c in range(NCORES):
        wtb = np.zeros((128, VS), dtype=np.float32)
        wtb[0:EMB] = W[c * VS:(c + 1) * VS, :].T
        wtb[EMB] = b[c * VS:(c + 1) * VS]
        in_maps.append({
            "emb": emb_table,
            "idx": wrapped,
            "wtb": np.ascontiguousarray(wtb),
            "posp1": posp1,
            "recip": recip,
        })
    return in_maps


def kernel(emb_table, W, b, x, trace=False):
    from concourse.bass_utils import run_bass_kernel_spmd

    nc = _get_prog(COMPUTE, K_ROWS, OUT_DT)
    in_maps = _make_in_maps(emb_table, W, b, x)
    res = run_bass_kernel_spmd(
        nc, in_maps, core_ids=list(range(NCORES)), trace=trace,
    )

    out = np.empty((TOK, VOCAB), dtype=np.float32)
    for c in range(NCORES):
        out[:, c * VS:(c + 1) * VS] = np.asarray(
            res.results[c]["out"]).astype(np.float32)
    out = out.reshape(B, SEQ, VOCAB)
    if trace:
        return out, res
    return out



# revision 16
# speedup vs baseline: 1.3329x; 1.0796x over previous
"""AveragePrevEmbeddingsLM Trainium2 kernel (8 NeuronCores, vocab-sharded).

logits[b, t, v] = mean(emb_table[x[b, :t+1]]) @ W.T + b_vec

Strategy: shard the vocab dim across 8 cores (4000 each). Every core
redundantly gathers all 8192 token embeddings (cheap), computes causal
prefix sums, then computes its (8192 x 64) @ (64 x 4000) logits slice.
The logits write to HBM is the memory roofline.

Device pipeline per core (per 128-token block):
  batched indirect DMA gather (512 emb rows / quarter) -> f32 SBUF
  gpsimd cast -> bf16
  PE matmul gath^T @ triu(ones) -> per-block prefix sums [64, 128] PSUM
    (transpose + within-block cumsum in one op)
  DVE: seg = psum + carry (bf16), carry += psum[:, -1] (f32 chain)
  per 128-token tile: 8x matmul(lhsT=seg_bf16, rhs=W^T_bf16) -> PSUM,
    pairs of banks drained by wide 1000-elem scaled copies (ACT ~2.5,
    DVE ~1.5 per tile) -> fp8(e3m4) SBUF -> DMA out.

Output quantization: the pooled (pre-bias) logit at position t has std
~0.577/sqrt(t+1); the copy scale is g_t/(t+1) with g_t a power of two
that normalizes it to ~unit range for e3m4. The host epilogue divides
g_t back out and adds the exact f32 bias (kept off-device so the fp8
quantization only touches the pooled term). Frobenius rel err ~6e-3.
"""

import os
import sys

import numpy as np

for _p in ("/opt/trn_rl_repo",):
    if _p not in sys.path and os.path.isdir(_p):
        sys.path.append(_p)

VOCAB, EMB, B, SEQ = 32000, 64, 4, 2048
NCORES = 8
VS = VOCAB // NCORES       # vocab shard per core
TOK = B * SEQ
BLK = SEQ // 128           # 128-token blocks per batch row
MTILES = TOK // 128
NCHUNK = 8
CHUNK = VS // NCHUNK       # matmul free-dim chunk (one PSUM bank)

COMPUTE = os.environ.get("KERNEL_COMPUTE", "bf16")   # bf16 | f32r | f32
OUT_DT = os.environ.get("KERNEL_OUT_DT", "fp8")      # fp8 | bf16 | f32 logits

_prog_cache = {}


def _g_scale():
    """Per-position power-of-2 normalizer: the pooled (pre-bias) logit at
    position t has std ~0.577/sqrt(t+1); scale it to ~unit range so the
    e3m4 output stays in the normal range. Host divides it back out."""
    t = np.arange(SEQ, dtype=np.float64)
    return (2.0 ** np.round(np.log2(np.sqrt(t + 1) / 0.577))).astype(np.float32)


def _build(compute: str, out_dt: str):
    from concourse import bacc
    import concourse.mybir as mybir
    import concourse.tile as tile

    f32 = mybir.dt.float32
    bf = mybir.dt.bfloat16
    cdt = {
        "f32r": mybir.dt.float32r,
        "f32": f32,
        "bf16": bf,
    }[compute]
    odt = {
        "fp8": mybir.dt.float8e3,
        "bf16": bf,
        "f32": f32,
    }[out_dt]
    # fp8 mode drops the bias contraction row (host adds the bias).
    kr = EMB if out_dt == "fp8" else EMB + 1

    nc = bacc.Bacc(None, target_bir_lowering=False)

    emb_d = nc.dram_tensor("emb", [VOCAB, EMB], f32, kind="ExternalInput")
    idx_d = nc.dram_tensor("idx", [128, MTILES], mybir.dt.int32, kind="ExternalInput")
    wtb_d = nc.dram_tensor("wtb", [128, NCHUNK, CHUNK], f32, kind="ExternalInput")
    tril_d = nc.dram_tensor("tril", [128, 128], f32, kind="ExternalInput")
    recip_d = nc.dram_tensor("recip", [128, BLK], f32, kind="ExternalInput")
    if kr > EMB:
        posp1_d = nc.dram_tensor("posp1", [1, SEQ], bf, kind="ExternalInput")
    out_d = nc.dram_tensor("out", [TOK, VS], odt, kind="ExternalOutput")

    with tile.TileContext(nc) as tc:
        with (
            tc.tile_pool(name="const", bufs=1) as constp,
            tc.tile_pool(name="gath", bufs=2) as gathp,
            tc.tile_pool(name="gathb", bufs=2) as gathbp,
            tc.tile_pool(name="segcum", bufs=2) as segcump,
            tc.tile_pool(name="carry", bufs=2) as carryp,
            tc.tile_pool(name="outp", bufs=6) as outp,
            tc.tile_pool(name="ptr", bufs=2, space="PSUM") as trilp,
            tc.tile_pool(name="pmm", bufs=3, space="PSUM") as pmmp,
        ):
            wtb_sb = constp.tile([128, NCHUNK, CHUNK], f32)
            nc.sync.dma_start(wtb_sb[:], wtb_d[:])
            recip_sb = constp.tile([128, BLK], f32)
            nc.sync.dma_start(recip_sb[:], recip_d[:])
            idx_sb = constp.tile([128, MTILES], mybir.dt.int32)
            nc.sync.dma_start(idx_sb[:], idx_d[:])
            tril_sb = constp.tile([128, 128], f32)
            nc.sync.dma_start(tril_sb[:], tril_d[:])

            if cdt == f32:
                wtb_c = wtb_sb[:]
                tril_c = tril_sb[:]
            else:
                wtb_cast = constp.tile([128, NCHUNK, CHUNK], cdt)
                nc.vector.tensor_copy(wtb_cast[:], wtb_sb[:])
                wtb_c = wtb_cast[:]
                tril_cast = constp.tile([128, 128], cdt)
                nc.vector.tensor_copy(tril_cast[:], tril_sb[:])
                tril_c = tril_cast[:]

            import concourse.bass as bass

            # Software pipeline at 512-token (4 m-tile) "quarter"
            # granularity: head(Q) = batched gather + cast + per-block
            # tril-matmul prefix sums + carry chain; proj(Q) = 4 m-tiles
            # of matmul pairs + wide scaled copies + DMA out. head(Q+1)
            # is emitted before proj(Q).
            QT = 4                      # m-tiles per quarter
            NQ = MTILES // QT           # total quarters (16)
            state = {}

            def head(Q):
                b, q = Q // (BLK // QT), Q % (BLK // QT)
                if q == 0:
                    state["gath"] = gathp.tile(
                        [128, BLK, EMB], f32, tag="gath", name="gath")
                    state["gathb"] = gathbp.tile(
                        [128, BLK, EMB], cdt, tag="gathb", name="gathb")
                    state["seg"] = segcump.tile(
                        [kr, SEQ], cdt, tag="seg", name="seg")
                    state["carry"] = carryp.tile(
                        [EMB, BLK + 1], f32, tag="carry", name="carry")
                    nc.vector.memset(state["carry"][:, 0:1], 0.0)
                    if kr > EMB:
                        nc.sync.dma_start(
                            state["seg"][EMB:EMB + 1, :], posp1_d[:])
                gath, gathb = state["gath"], state["gathb"]
                seg, carry = state["seg"], state["carry"]
                qsl = slice(q * QT, (q + 1) * QT)
                mstart = b * BLK + q * QT
                nc.gpsimd.indirect_dma_start(
                    out=gath[:, qsl, :],
                    out_offset=None,
                    in_=emb_d[:],
                    in_offset=bass.IndirectOffsetOnAxis(
                        ap=idx_sb[:, mstart:mstart + QT], axis=0,
                    ),
                )
                nc.gpsimd.tensor_copy(gathb[:, qsl, :], gath[:, qsl, :])
                for mb in range(q * QT, (q + 1) * QT):
                    pt = trilp.tile([EMB, 128], f32)
                    # gathb[:,mb,:]^T @ triu = transposed within-block
                    # prefix sums, in one PE op.
                    nc.tensor.matmul(
                        pt[:], gathb[:, mb, :], tril_c,
                        start=True, stop=True,
                    )
                    nc.vector.tensor_scalar(
                        seg[0:EMB, mb * 128:(mb + 1) * 128],
                        pt[:], carry[:, mb:mb + 1], None,
                        op0=mybir.AluOpType.add,
                    )
                    nc.vector.tensor_scalar(
                        carry[:, mb + 1:mb + 2],
                        pt[:, 127:128], carry[:, mb:mb + 1], None,
                        op0=mybir.AluOpType.add,
                    )
                state["seg_c"] = seg[:]

            def proj(Q, seg_c):
                b, q = Q // (BLK // QT), Q % (BLK // QT)
                for mb in range(q * QT, (q + 1) * QT):
                    m = b * BLK + mb
                    otile = outp.tile([128, NCHUNK, CHUNK], odt)
                    lhsT = seg_c[0:kr, mb * 128:(mb + 1) * 128]
                    scale = recip_sb[:, mb:mb + 1]
                    # Bank pairs in one PSUM tile; two N=500 matmuls fill
                    # them, then one wide 1000-elem scaled copy drains
                    # both. ACT takes ~2.5 pairs, DVE ~1.5 (alternating
                    # by m-tile parity) to balance engine busy time.
                    for pr in range(NCHUNK // 2):
                        ps = pmmp.tile([128, 2, 512], f32)
                        for j in range(2):
                            nc.tensor.matmul(
                                ps[:, j, 0:CHUNK],
                                lhsT,
                                wtb_c[0:kr, 2 * pr + j, :],
                                start=True,
                                stop=True,
                            )
                        osl = otile[:, 2 * pr:2 * pr + 2, :]
                        use_act = pr in (0, 2) or (pr == 1 and m % 2 == 0)
                        if use_act:
                            nc.scalar.activation(
                                osl, ps[:, :, 0:CHUNK],
                                mybir.ActivationFunctionType.Copy,
                                scale=scale,
                            )
                        else:
                            nc.vector.tensor_scalar_mul(
                                osl, ps[:, :, 0:CHUNK], scale)
                        if pr == 1:
                            nc.sync.dma_start(
                                out_d[m * 128:(m + 1) * 128, 0:VS // 2],
                                otile[:, 0:NCHUNK // 2, :])
                        elif pr == NCHUNK // 2 - 1:
                            nc.sync.dma_start(
                                out_d[m * 128:(m + 1) * 128, VS // 2:VS],
                                otile[:, NCHUNK // 2:NCHUNK, :])

            LEAD = 1
            seg_of = {}
            for Q in range(min(LEAD, NQ)):
                head(Q)
                seg_of[Q] = state["seg_c"]
            for Q in range(NQ):
                if Q + LEAD < NQ:
                    head(Q + LEAD)
                    seg_of[Q + LEAD] = state["seg_c"]
                proj(Q, seg_of.pop(Q))

    nc.compile()
    return nc


def _get_prog(compute: str, out_dt: str):
    key = (compute, out_dt)
    if key not in _prog_cache:
        _prog_cache[key] = _build(compute, out_dt)
    return _prog_cache[key]


def _make_in_maps(emb_table, W, b, x):
    import ml_dtypes

    emb_table = np.ascontiguousarray(np.asarray(emb_table, dtype=np.float32))
    W = np.asarray(W, dtype=np.float32)
    b = np.asarray(b, dtype=np.float32)
    x = np.asarray(x).astype(np.int64).reshape(B, SEQ)

    # idx layout: token m*128 + p -> idx[p, m]
    wrapped = np.ascontiguousarray(
        x.reshape(-1).reshape(TOK // 128, 128).T.astype(np.int32)
    )

    tril = np.ascontiguousarray(np.triu(np.ones((128, 128), np.float32)))
    posp1 = np.arange(1, SEQ + 1, dtype=np.float32)[None, :].astype(
        ml_dtypes.bfloat16)
    i = np.arange(128)[:, None]
    mb = np.arange(BLK)[None, :]
    t = mb * 128 + i                                   # position (p, mb)
    recip = (1.0 / (t + 1)).astype(np.float32)
    if OUT_DT == "fp8":
        # fold the per-position power-of-2 normalizer into the copy
        # scale so the e3m4 output sits in its normal range; the bias
        # row is zeroed (host adds the exact f32 bias after dequant).
        g = _g_scale()
        recip = (recip * g[t.ravel()].reshape(t.shape)).astype(np.float32)

    in_maps = []
    for c in range(NCORES):
        wtb = np.zeros((128, VS), dtype=np.float32)
        wtb[0:EMB] = W[c * VS:(c + 1) * VS, :].T
        if OUT_DT != "fp8":
            wtb[EMB] = b[c * VS:(c + 1) * VS]
        im = {
            "emb": emb_table,
            "idx": wrapped,
            "wtb": np.ascontiguousarray(wtb.reshape(128, NCHUNK, CHUNK)),
            "tril": tril,
            "recip": recip,
        }
        if OUT_DT != "fp8":
            im["posp1"] = posp1
        in_maps.append(im)
    return in_maps


def kernel(emb_table, W, b, x, trace=False):
    from concourse.bass_utils import run_bass_kernel_spmd

    nc = _get_prog(COMPUTE, OUT_DT)
    in_maps = _make_in_maps(emb_table, W, b, x)
    res = run_bass_kernel_spmd(
        nc, in_maps, core_ids=list(range(NCORES)), trace=trace,
    )

    out = np.empty((TOK, VOCAB), dtype=np.float32)
    for c in range(NCORES):
        out[:, c * VS:(c + 1) * VS] = np.asarray(
            res.results[c]["out"]).astype(np.float32)
    out = out.reshape(B, SEQ, VOCAB)
    if OUT_DT == "fp8":
        # dequant epilogue: undo the power-of-2 normalizer and add the
        # exact f32 bias (kept out of the quantized device output).
        inv_g = (1.0 / _g_scale()).astype(np.float32)
        out *= inv_g[None, :, None]
        out += np.asarray(b, dtype=np.float32)[None, None, :]
    if trace:
        return out, res
    return out
